# revision 1
# baseline (speedup 1.0000x reference)
"""Multi-head attention on 8 TRN2 NeuronCores.

Sharding: core c -> batch b = c//2, head-group g = c%2 (8 of 16 heads).
Each core computes, for its (batch, 8 heads):
    Q^T/K^T projections (head-dim on partitions), V natural layout,
    transposed scores S^T[t,s] per head, exp on ACT, unnormalized AV^T with
    a ones-column in V producing the softmax denominator row, normalization
    via a K=1 broadcast matmul + fast reciprocal, and the partial output
    projection against this head-group's 512 rows of Wo.
Host side: inputs are pre-transposed/cast/packed per core (bf16), the two
head-group partials per batch are summed and bo added (fp32).

Row masking (scores rows s >= len zeroed pre-softmax) is folded in for
free: masked columns of x_Q^T are zeroed on the host and the Q bias is
injected via a K=1 matmul against the mask row, so masked queries get
Q[s]=0 -> uniform softmax rows, exactly matching the reference.

The emission order software-pipelines the (in-order) PE stream so matmuls
never sit behind a wait for ACT's exp: Q/K pair projections and V are
interleaved with scores of earlier heads, and uav(h) is emitted two head
slots after scores(h).
"""

import sys

sys.path.insert(0, "/opt/trn_rl_repo")

import numpy as np
import ml_dtypes

B, S, D, H, DH = 4, 1024, 1024, 16, 64
P = 128
NPAIR = 4  # head pairs per core (8 heads)
SCALE = 1.0 / 8.0  # 1/sqrt(DH), folded into Wq/bq on host

_CACHED = None


def _build():
    import concourse.bass as bass
    import concourse.mybir as mybir
    from concourse.tile import TileContext

    bf16 = mybir.dt.bfloat16
    f32 = mybir.dt.float32
    Exp = mybir.ActivationFunctionType.Exp

    nc = bass.Bass()
    xq = nc.dram_tensor("xq", [D, S], bf16, kind="ExternalInput")  # x_Q[b].T, masked cols zeroed
    xk = nc.dram_tensor("xk", [D, S], bf16, kind="ExternalInput")
    xv = nc.dram_tensor("xv", [D, S], bf16, kind="ExternalInput")
    wq = nc.dram_tensor("wq", [D, 512], bf16, kind="ExternalInput")  # pre-scaled
    wk = nc.dram_tensor("wk", [D, 512], bf16, kind="ExternalInput")
    wv = nc.dram_tensor("wv", [D, 512], bf16, kind="ExternalInput")
    wo = nc.dram_tensor("wo", [512, D], bf16, kind="ExternalInput")
    bqc = nc.dram_tensor("bq", [1, 512], f32, kind="ExternalInput")  # pre-scaled
    bkc = nc.dram_tensor("bk", [1, 512], f32, kind="ExternalInput")
    bv = nc.dram_tensor("bv", [1, 512], bf16, kind="ExternalInput")
    mask = nc.dram_tensor("mask", [1, S], bf16, kind="ExternalInput")
    out = nc.dram_tensor("out", [S, D], f32, kind="ExternalOutput")

    with TileContext(nc) as tc:
        with (
            tc.tile_pool(name="persist", bufs=1) as persist,
            tc.tile_pool(name="expp", bufs=2) as expp,
            tc.tile_pool(name="small", bufs=4) as small,
            tc.tile_pool(name="outp", bufs=2) as outp,
            tc.tile_pool(name="stagep", bufs=8) as stagep,
            tc.tile_pool(name="ps", bufs=4, space="PSUM") as psp,
            tc.tile_pool(name="ps2", bufs=2, space="PSUM") as psp2,
        ):
            def ps_tile():
                return psp.tile([P, 512], f32, tag="ps", name="ps")

            def sc_tile():
                return psp2.tile([P, 1024], f32, tag="sc", name="sc")

            # ---- constants and small rows first ----
            bv_sb = persist.tile([1, 512], bf16, tag="bv")
            mask_sb = persist.tile([1, S], bf16, tag="mask")
            nc.sync.dma_start(bv_sb[:], bv[:])
            nc.sync.dma_start(mask_sb[:], mask[:])
            ones_sb = persist.tile([1, 512], bf16, tag="ones")
            nc.vector.memset(ones_sb[:], 1.0)
            bqc_sb = persist.tile([P, 4], f32, tag="bqc")
            bkc_sb = persist.tile([P, 4], f32, tag="bkc")
            nc.sync.dma_start(
                bqc_sb[:], bqc.rearrange("o (c p) -> p c o", p=P)[:, :, 0]
            )
            nc.sync.dma_start(
                bkc_sb[:], bkc.rearrange("o (c p) -> p c o", p=P)[:, :, 0]
            )
            mask_bc = persist.tile([P, S], bf16, tag="mask_bc")

            # weight/x tiles; DMA chunked by d-chunk so matmuls start early
            xq_sb = persist.tile([P, 8, S], bf16, tag="xq")
            xk_sb = persist.tile([P, 8, S], bf16, tag="xk")
            xv_sb = persist.tile([P, 8, S], bf16, tag="xv")
            wq_sb = persist.tile([P, 8, 512], bf16, tag="wq")
            wk_sb = persist.tile([P, 8, 512], bf16, tag="wk")
            wv_sb = persist.tile([P, 8, 512], bf16, tag="wv")
            xq_r = xq.rearrange("(c p) s -> p c s", p=P)
            xk_r = xk.rearrange("(c p) s -> p c s", p=P)
            xv_r = xv.rearrange("(c p) s -> p c s", p=P)
            wq_r = wq.rearrange("(c p) m -> p c m", p=P)
            wk_r = wk.rearrange("(c p) m -> p c m", p=P)
            wv_r = wv.rearrange("(c p) m -> p c m", p=P)
            for dc in range(8):
                nc.sync.dma_start(wq_sb[:, dc, :], wq_r[:, dc, :])
                nc.scalar.dma_start(wk_sb[:, dc, :], wk_r[:, dc, :])
                nc.sync.dma_start(xq_sb[:, dc, :], xq_r[:, dc, :])
                nc.scalar.dma_start(xk_sb[:, dc, :], xk_r[:, dc, :])
            for dc in range(8):
                nc.scalar.dma_start(wv_sb[:, dc, :], wv_r[:, dc, :])
                nc.scalar.dma_start(xv_sb[:, dc, :], xv_r[:, dc, :])

            QT = [persist.tile([P, S], bf16, tag=f"qt{p}", name=f"qt{p}") for p in range(NPAIR)]
            KT = [persist.tile([P, S], bf16, tag=f"kt{p}", name=f"kt{p}") for p in range(NPAIR)]
            AVT = [persist.tile([P, S], bf16, tag=f"avt{p}", name=f"avt{p}") for p in range(NPAIR)]
            vaug = persist.tile([P, 8, 8 * 65], bf16, tag="vaug")
            nc.vector.memset(vaug[:], 1.0)

            expS = [None] * 8
            # denominator rows, repacked [s] -> [128 partitions, 8/partition] so
            # the reciprocal runs partition-parallel, then flattened back to a
            # [1, 8, S] row tile for the K=1 broadcast matmuls.
            stages = [None] * 8
            packed = persist.tile([P, 8, 8], f32, tag="packed")
            packed_b = persist.tile([P, 8, 8], bf16, tag="packedb")
            rdrow = persist.tile([1, 8, S], bf16, tag="rdrow")

            def emit_qk_pair(p):
                for w_sb, x_sb, b_col, dstT, masked in (
                    (wq_sb, xq_sb, bqc_sb, QT, True),
                    (wk_sb, xk_sb, bkc_sb, KT, False),
                ):
                    for st in range(2):
                        ps = ps_tile()
                        for dc in range(8):
                            nc.tensor.matmul(
                                ps[:],
                                lhsT=w_sb[:, dc, p * P : (p + 1) * P],
                                rhs=x_sb[:, dc, st * 512 : (st + 1) * 512],
                                start=(dc == 0),
                                stop=(dc == 7),
                            )
                        dst = dstT[p][:, st * 512 : (st + 1) * 512]
                        if masked:
                            nc.vector.scalar_tensor_tensor(
                                dst,
                                ps[:],
                                b_col[:, p : p + 1],
                                mask_bc[:, st * 512 : (st + 1) * 512],
                                mybir.AluOpType.add,
                                mybir.AluOpType.mult,
                            )
                        else:
                            nc.vector.tensor_scalar_add(
                                dst, ps[:], b_col[:, p : p + 1]
                            )

            def emit_v(tcn):
                ps = ps_tile()
                nc.tensor.matmul(
                    ps[:],
                    lhsT=ones_sb[0:1, 0:P],
                    rhs=bv_sb[0:1, 0:512],
                    start=True,
                    stop=False,
                )
                for dc in range(8):
                    nc.tensor.matmul(
                        ps[:],
                        lhsT=xv_sb[:, dc, tcn * P : (tcn + 1) * P],
                        rhs=wv_sb[:, dc, 0:512],
                        start=False,
                        stop=(dc == 7),
                    )
                nc.vector.tensor_copy(
                    vaug[:, tcn, :].rearrange("p (h x) -> p h x", x=65)[:, :, 0:64],
                    ps[:].rearrange("p (h v) -> p h v", v=64),
                )

            def emit_scores(h):
                p, base = h // 2, 64 * (h % 2)
                expS[h] = expp.tile([P, 8, S], bf16, tag="expS", name="expS")
                for tcn in range(8):
                    ps = sc_tile()
                    for st in range(2):
                        nc.tensor.matmul(
                            ps[:, st * 512 : (st + 1) * 512],
                            lhsT=KT[p][base : base + 64, tcn * P : (tcn + 1) * P],
                            rhs=QT[p][base : base + 64, st * 512 : (st + 1) * 512],
                            start=True,
                            stop=True,
                        )
                    nc.scalar.activation(expS[h][:, tcn, :], ps[:], Exp)

            def emit_uav(h):
                p, base = h // 2, 64 * (h % 2)
                stage = stagep.tile([1, S], f32, tag="stage", name="stage")
                stages[h] = stage
                for st in range(2):
                    psu = ps_tile()
                    for tcn in range(8):
                        nc.tensor.matmul(
                            psu[0:65, :],
                            lhsT=vaug[:, tcn, h * 65 : (h + 1) * 65],
                            rhs=expS[h][:, tcn, st * 512 : (st + 1) * 512],
                            start=(tcn == 0),
                            stop=(tcn == 7),
                        )
                    nc.vector.tensor_copy(
                        AVT[p][base : base + 64, st * 512 : (st + 1) * 512],
                        psu[0:64, :],
                    )
                    nc.vector.tensor_copy(
                        stage[0:1, st * 512 : (st + 1) * 512], psu[64:65, :]
                    )
            def emit_pack(h):
                nc.sync.dma_start(
                    packed[:, h, :],
                    stages[h][0:1, :].rearrange("o (p j) -> o p j", j=8),
                )

            def emit_recip(i):
                hs = slice(4 * i, 4 * i + 4)
                nc.vector.reciprocal(packed[:, hs, :], packed[:, hs, :])
                nc.vector.tensor_copy(packed_b[:, hs, :], packed[:, hs, :])
                for h in range(4 * i, 4 * i + 4):
                    nc.sync.dma_start(
                        rdrow[0:1, h, :].rearrange("o (p j) -> o p j", j=8),
                        packed_b[:, h, :],
                    )

            def emit_norm(h):
                # AVT[h] *= 1/denom[h,s], broadcast across the 64 v-partitions
                # via a K=1 outer-product matmul of the reciprocal row.
                p, base = h // 2, 64 * (h % 2)
                for st in range(2):
                    psr = ps_tile()
                    nc.tensor.matmul(
                        psr[0:64, :],
                        lhsT=ones_sb[0:1, 0:64],
                        rhs=rdrow[0:1, h, st * 512 : (st + 1) * 512],
                        start=True,
                        stop=True,
                    )
                    av = AVT[p][base : base + 64, st * 512 : (st + 1) * 512]
                    nc.vector.tensor_mul(av, av, psr[0:64, :])

            # mask broadcast [128, S] for the fused Q bias+mask epilogue
            for st in range(2):
                psm = ps_tile()
                nc.tensor.matmul(
                    psm[:],
                    lhsT=ones_sb[0:1, 0:P],
                    rhs=mask_sb[0:1, st * 512 : (st + 1) * 512],
                    start=True,
                    stop=True,
                )
                nc.vector.tensor_copy(mask_bc[:, st * 512 : (st + 1) * 512], psm[:])

            # ---- software-pipelined emission ----
            emit_qk_pair(0)
            emit_scores(0)
            emit_qk_pair(1)
            emit_scores(1)
            for tcn in range(8):
                emit_v(tcn)
            emit_qk_pair(2)
            emit_scores(2)
            emit_uav(0)
            emit_pack(0)
            emit_qk_pair(3)
            emit_scores(3)
            emit_uav(1)
            emit_pack(1)
            emit_scores(4)
            emit_uav(2)
            emit_pack(2)
            emit_scores(5)
            emit_uav(3)
            emit_pack(3)
            emit_recip(0)
            emit_scores(6)
            emit_uav(4)
            emit_pack(4)
            emit_scores(7)
            emit_uav(5)
            emit_pack(5)
            emit_uav(6)
            emit_pack(6)
            emit_uav(7)
            emit_pack(7)
            emit_recip(1)
            for h in range(8):
                emit_norm(h)

            # ---- output projection: out[s(128/chunk), m] ----
            wo_sb = expp.tile([P, 4, D], bf16, tag="expS", name="wo_sb")
            nc.sync.dma_start(wo_sb[:], wo.rearrange("(c p) m -> p c m", p=P))
            for sc in range(8):
                osb = outp.tile([P, D], f32, tag="osb", name="osb")
                for mt in range(2):
                    ps = ps_tile()
                    for p in range(NPAIR):
                        nc.tensor.matmul(
                            ps[:],
                            lhsT=AVT[p][:, sc * P : (sc + 1) * P],
                            rhs=wo_sb[:, p, mt * 512 : (mt + 1) * 512],
                            start=(p == 0),
                            stop=(p == NPAIR - 1),
                        )
                    nc.vector.tensor_copy(osb[:, mt * 512 : (mt + 1) * 512], ps[:])
                nc.sync.dma_start(out[sc * P : (sc + 1) * P, :], osb[:])

    _split_multiwait(nc)
    return nc


def _split_multiwait(nc):
    """This container's walrus rejects >1 sync wait on CTRL-class
    instructions (Tile's exit Drain carries one per outstanding proc).
    Hoist all but the last wait onto preceding same-engine NoOps."""
    import concourse.mybir as mybir

    for f in nc.m.functions:
        for bb in f.blocks:
            insts = list(bb.instructions)
            res, changed = [], False
            for inst in insts:
                si = inst.sync_info
                waits = list(si.on_wait) if si is not None else []
                if len(waits) > 1:
                    for w in waits[:-1]:
                        res.append(
                            mybir.InstNoOp(
                                name=nc.get_next_instruction_name(),
                                sync_info=mybir.SyncInfo(on_wait=[w], on_update=[]),
                                bass_nofuse=True,
                                engine=inst.engine,
                            )
                        )
                    inst.sync_info = mybir.SyncInfo(
                        on_wait=[waits[-1]], on_update=list(si.on_update)
                    )
                    changed = True
                res.append(inst)
            if changed:
                bb.instructions = res


def _shard_inputs(x_Q, x_K, x_V, src_batch_lens, Wq, bq, Wk, bk, Wv, bv, Wo, bo):
    bf = ml_dtypes.bfloat16
    f32 = np.float32
    in_maps = []
    # head-major packed weights [D, H*DH] and biases [1, H*DH]
    wq_all = (np.asarray(Wq, f32).transpose(1, 0, 2).reshape(D, H * DH) * SCALE).astype(bf)
    wk_all = np.asarray(Wk, f32).transpose(1, 0, 2).reshape(D, H * DH).astype(bf)
    wv_all = np.asarray(Wv, f32).transpose(1, 0, 2).reshape(D, H * DH).astype(bf)
    bq_all = (np.asarray(bq, f32).reshape(1, H * DH) * SCALE).astype(f32)
    bk_all = np.asarray(bk, f32).reshape(1, H * DH).astype(f32)
    bv_all = np.asarray(bv, f32).reshape(1, H * DH).astype(bf)
    wo_bf = np.asarray(Wo, f32).astype(bf)
    for c in range(8):
        b, g = c // 2, c % 2
        ln = int(src_batch_lens[b])
        m = (np.arange(S) < ln).astype(f32)
        xqT = np.ascontiguousarray(np.asarray(x_Q[b], f32).T * m[None, :]).astype(bf)
        xkT = np.ascontiguousarray(np.asarray(x_K[b], f32).T).astype(bf)
        xvT = np.ascontiguousarray(np.asarray(x_V[b], f32).T).astype(bf)
        hs = slice(g * 512, (g + 1) * 512)
        in_maps.append(
            {
                "xq": xqT,
                "xk": xkT,
                "xv": xvT,
                "wq": np.ascontiguousarray(wq_all[:, hs]),
                "wk": np.ascontiguousarray(wk_all[:, hs]),
                "wv": np.ascontiguousarray(wv_all[:, hs]),
                "wo": np.ascontiguousarray(wo_bf[hs, :]),
                "bq": np.ascontiguousarray(bq_all[:, hs]),
                "bk": np.ascontiguousarray(bk_all[:, hs]),
                "bv": np.ascontiguousarray(bv_all[:, hs]),
                "mask": m.reshape(1, S).astype(bf),
            }
        )
    return in_maps


def kernel(**inputs):
    global _CACHED
    from concourse.bass_utils import run_bass_kernel_spmd

    if _CACHED is None:
        _CACHED = _build()
    nc = _CACHED
    in_maps = _shard_inputs(**inputs)
    res = run_bass_kernel_spmd(nc, in_maps, core_ids=list(range(8)))
    bo = np.asarray(inputs["bo"], np.float32)
    out = np.empty((B, S, D), np.float32)
    for b in range(B):
        out[b] = res.results[2 * b]["out"] + res.results[2 * b + 1]["out"] + bo[None, :]
    return out



# revision 4
# speedup vs baseline: 1.2281x; 1.2281x over previous
"""Multi-head attention on 8 TRN2 NeuronCores.

Sharding: core c -> (batch-pair p = c//4, head-quarter q = c%4).
Each core handles 4 heads x 2 batches. Queries are PACKED on the host:
only the first len_b valid query columns plus one zero column (whose
softmax row is uniform -> reproduces the reference's masked rows) are
shipped; the host scatters/broadcasts rows back afterwards. Batches are
paired (largest query count with smallest) so the single SPMD program,
compiled for the unified per-slot counts (NA, NB), wastes little work.

Per core, transposed-attention layout as before:
  Q^T/K^T projections (head-dim on partitions), V in natural layout
  augmented with a ones column per head (softmax denominator rides the
  65th row of the AV^T matmul), scores^T per head with the two heads of
  a 128-row pair issued back-to-back to opposite 64-row PE row-groups
  (they execute concurrently), exp on ACT writing fp8e4 probabilities,
  unnormalized AV^T, reciprocal + K=1 broadcast matmul normalization,
  and the partial output projection against this quarter's 256 rows of
  Wo. Host sums the 4 quarter-partials per batch and adds bo.

K-projection bias epilogues run on ACT (activation Identity with a
per-partition bias column); Q (bias+mask) and V epilogues on DVE.
"""

import sys

sys.path.insert(0, "/opt/trn_rl_repo")

import numpy as np
import ml_dtypes

B, S, D, H, DH = 4, 1024, 1024, 16, 64
P = 128
SCALE = 1.0 / 8.0  # 1/sqrt(DH), folded into wq/bq on host

_CACHED = None  # last-built program (test.py compatibility)
_CACHE = {}
_PLAN = None


def _tiles(total, step):
    out = []
    off = 0
    while off < total:
        n = min(step, total - off)
        out.append((off, n))
        off += n
    return out


def _build(NA, NB):
    import concourse.bass as bass
    import concourse.mybir as mybir
    from concourse.tile import TileContext

    bf16 = mybir.dt.bfloat16
    f32 = mybir.dt.float32
    fp8 = mybir.dt.float8e4
    Exp = mybir.ActivationFunctionType.Exp
    Ident = mybir.ActivationFunctionType.Identity

    NQ = NA + NB
    NQP = ((NQ + P - 1) // P) * P
    JP = NQP // P  # packed denominator columns per partition

    nc = bass.Bass()
    xq = nc.dram_tensor("xq", [D, NQ], bf16, kind="ExternalInput")
    xk = nc.dram_tensor("xk", [D, 2, S], bf16, kind="ExternalInput")
    xv = nc.dram_tensor("xv", [D, 2, S], bf16, kind="ExternalInput")
    wq = nc.dram_tensor("wq", [D, 256], bf16, kind="ExternalInput")  # pre-scaled
    wk = nc.dram_tensor("wk", [D, 256], bf16, kind="ExternalInput")
    wv = nc.dram_tensor("wv", [D, 256], bf16, kind="ExternalInput")
    wo = nc.dram_tensor("wo", [256, D], bf16, kind="ExternalInput")
    bqc = nc.dram_tensor("bq", [1, 256], f32, kind="ExternalInput")  # pre-scaled
    bkc = nc.dram_tensor("bk", [1, 256], f32, kind="ExternalInput")
    bv = nc.dram_tensor("bv", [1, 256], bf16, kind="ExternalInput")
    mask = nc.dram_tensor("mask", [1, NQ], bf16, kind="ExternalInput")
    out = nc.dram_tensor("out", [NQ, D], bf16, kind="ExternalOutput")

    QOFF = (0, NA)  # query-column offset per batch slot
    NB_ = (NA, NB)

    with TileContext(nc) as tc:
        with (
            tc.tile_pool(name="persist", bufs=1) as persist,
            tc.tile_pool(name="expp", bufs=2) as expp,
            tc.tile_pool(name="outp", bufs=2) as outp,
            tc.tile_pool(name="ps", bufs=4, space="PSUM") as psp,
            tc.tile_pool(name="sc", bufs=2, space="PSUM") as scp,
        ):
            # ---- small constants ----
            bv_sb = persist.tile([1, 256], bf16, tag="bv")
            mask_sb = persist.tile([1, NQ], bf16, tag="mask")
            nc.sync.dma_start(bv_sb[:], bv[:])
            nc.sync.dma_start(mask_sb[:], mask[:])
            ones_sb = persist.tile([1, 512], bf16, tag="ones")
            nc.vector.memset(ones_sb[:], 1.0)
            bqc_sb = persist.tile([P, 2], f32, tag="bqc")
            bkc_sb = persist.tile([P, 2], f32, tag="bkc")
            nc.sync.dma_start(bqc_sb[:], bqc.rearrange("o (c p) -> p c o", p=P)[:, :, 0])
            nc.sync.dma_start(bkc_sb[:], bkc.rearrange("o (c p) -> p c o", p=P)[:, :, 0])
            mask_bc = persist.tile([P, NQ], bf16, tag="mask_bc")

            # ---- big inputs, chunked by d-chunk so matmuls start early ----
            xq_sb = persist.tile([P, 8, NQ], bf16, tag="xq")
            xk_sb = persist.tile([P, 8, 2 * S], bf16, tag="xk")
            xv_sb = persist.tile([P, 8, 2 * S], bf16, tag="xv")
            wq_sb = persist.tile([P, 8, 256], bf16, tag="wq")
            wk_sb = persist.tile([P, 8, 256], bf16, tag="wk")
            wv_sb = persist.tile([P, 8, 256], bf16, tag="wv")
            xq_r = xq.rearrange("(c p) s -> p c s", p=P)
            xk_r = xk.rearrange("(c p) b s -> p c b s", p=P)
            xv_r = xv.rearrange("(c p) b s -> p c b s", p=P)
            wq_r = wq.rearrange("(c p) m -> p c m", p=P)
            wk_r = wk.rearrange("(c p) m -> p c m", p=P)
            wv_r = wv.rearrange("(c p) m -> p c m", p=P)
            xk_v = xk_sb[:].rearrange("p c (b s) -> p c b s", b=2)
            xv_v = xv_sb[:].rearrange("p c (b s) -> p c b s", b=2)
            for dc in range(8):
                nc.sync.dma_start(wq_sb[:, dc, :], wq_r[:, dc, :])
                nc.gpsimd.dma_start(wk_sb[:, dc, :], wk_r[:, dc, :])
                nc.sync.dma_start(xq_sb[:, dc, :], xq_r[:, dc, :])
                nc.gpsimd.dma_start(xk_v[:, dc, 0, :], xk_r[:, dc, 0, :])
            for dc in range(8):
                nc.gpsimd.dma_start(xk_v[:, dc, 1, :], xk_r[:, dc, 1, :])
                nc.gpsimd.dma_start(wv_sb[:, dc, :], wv_r[:, dc, :])
                nc.gpsimd.dma_start(xv_v[:, dc, 0, :], xv_r[:, dc, 0, :])
                nc.gpsimd.dma_start(xv_v[:, dc, 1, :], xv_r[:, dc, 1, :])

            QT = [persist.tile([P, NQ], bf16, tag=f"qt{p}", name=f"qt{p}") for p in range(2)]
            KT = [persist.tile([P, 2, S], bf16, tag=f"kt{p}", name=f"kt{p}") for p in range(2)]
            AVT = [persist.tile([P, NQ], bf16, tag=f"avt{p}", name=f"avt{p}") for p in range(2)]
            vaug = persist.tile([P, 8, 2, 264], bf16, tag="vaug")
            nc.vector.memset(vaug[:], 1.0)

            exps = {}  # (pair, b) -> fp8 prob tile [P, 8, 2, N_b]
            stages = [persist.tile([1, NQP], f32, tag=f"stage{h}", name=f"stage{h}") for h in range(4)]
            for h in range(4):
                nc.vector.memset(stages[h][:], 1.0)
            packed = persist.tile([P, 4, JP], f32, tag="packed")
            packed_b = persist.tile([P, 4, JP], bf16, tag="packedb")
            rdrow = persist.tile([1, 4, NQP], bf16, tag="rdrow")

            def emit_maskbc():
                for off, n in _tiles(NQ, 512):
                    ps = psp.tile([P, 512], f32, tag="ps", name="ps")
                    nc.tensor.matmul(
                        ps[:, 0:n],
                        lhsT=ones_sb[0:1, 0:P],
                        rhs=mask_sb[0:1, off : off + n],
                        start=True,
                        stop=True,
                    )
                    nc.vector.tensor_copy(mask_bc[:, off : off + n], ps[:, 0:n])

            def emit_qproj(p):
                for off, n in _tiles(NQ, 512):
                    ps = psp.tile([P, 512], f32, tag="ps", name="ps")
                    for dc in range(8):
                        nc.tensor.matmul(
                            ps[:, 0:n],
                            lhsT=wq_sb[:, dc, p * P : (p + 1) * P],
                            rhs=xq_sb[:, dc, off : off + n],
                            start=(dc == 0),
                            stop=(dc == 7),
                        )
                    nc.vector.scalar_tensor_tensor(
                        QT[p][:, off : off + n],
                        ps[:, 0:n],
                        bqc_sb[:, p : p + 1],
                        mask_bc[:, off : off + n],
                        mybir.AluOpType.add,
                        mybir.AluOpType.mult,
                    )

            def emit_kproj(p, b):
                for off, n in _tiles(S, 512):
                    ps = psp.tile([P, 512], f32, tag="ps", name="ps")
                    for dc in range(8):
                        nc.tensor.matmul(
                            ps[:, 0:n],
                            lhsT=wk_sb[:, dc, p * P : (p + 1) * P],
                            rhs=xk_v[:, dc, b, off : off + n],
                            start=(dc == 0),
                            stop=(dc == 7),
                        )
                    nc.scalar.activation(
                        KT[p][:, b, off : off + n],
                        ps[:, 0:n],
                        Ident,
                        bias=bkc_sb[:, p : p + 1],
                    )

            def emit_vproj(b, tcn):
                ps = psp.tile([P, 512], f32, tag="ps", name="ps")
                nc.tensor.matmul(
                    ps[:, 0:256],
                    lhsT=ones_sb[0:1, 0:P],
                    rhs=bv_sb[0:1, 0:256],
                    start=True,
                    stop=False,
                )
                for dc in range(8):
                    nc.tensor.matmul(
                        ps[:, 0:256],
                        lhsT=xv_v[:, dc, b, tcn * P : (tcn + 1) * P],
                        rhs=wv_sb[:, dc, 0:256],
                        start=False,
                        stop=(dc == 7),
                    )
                nc.vector.tensor_copy(
                    vaug[:, tcn, b, :]
                    .rearrange("p (h x) -> p h x", x=66)[:, :, 0:64],
                    ps[:, 0:256].rearrange("p (h v) -> p h v", v=64),
                )

            def emit_scores_tcn(p, b, tcn):
                if (p, b) not in exps:
                    exps[(p, b)] = expp.tile(
                        [P, 8, 2, NB_[b]], fp8, tag=f"exps{b}", name=f"exps{b}"
                    )
                ex = exps[(p, b)]
                qo = QOFF[b]
                for off, n in _tiles(NB_[b], 512):
                    sc = scp.tile([P, 2, 512], f32, tag="sc", name="sc")
                    for hh in range(2):
                        nc.tensor.matmul(
                            sc[:, hh, 0:n],
                            lhsT=KT[p][hh * 64 : hh * 64 + 64, b, tcn * P : (tcn + 1) * P],
                            rhs=QT[p][hh * 64 : hh * 64 + 64, qo + off : qo + off + n],
                            start=True,
                            stop=True,
                        )
                    nc.scalar.activation(
                        ex[:, tcn, :, off : off + n], sc[:, :, 0:n], Exp
                    )

            def emit_uav(p, b, h):
                hh = h % 2
                ex = exps[(p, b)]
                qo = QOFF[b]
                for off, n in _tiles(NB_[b], 512):
                    ps = psp.tile([P, 512], f32, tag="ps", name="ps")
                    for tcn in range(8):
                        nc.tensor.matmul(
                            ps[0:65, 0:n],
                            lhsT=vaug[:, tcn, b, h * 66 : h * 66 + 65],
                            rhs=ex[:, tcn, hh, off : off + n],
                            start=(tcn == 0),
                            stop=(tcn == 7),
                        )
                    nc.vector.tensor_copy(
                        AVT[p][hh * 64 : hh * 64 + 64, qo + off : qo + off + n],
                        ps[0:64, 0:n],
                    )
                    nc.vector.tensor_copy(
                        stages[h][0:1, qo + off : qo + off + n], ps[64:65, 0:n]
                    )

            def emit_pack(h):
                nc.sync.dma_start(
                    packed[:, h, :],
                    stages[h][0:1, :].rearrange("o (p j) -> o p j", j=JP),
                )

            def emit_recip(i):
                hs = slice(2 * i, 2 * i + 2)
                nc.vector.reciprocal(packed[:, hs, :], packed[:, hs, :])
                nc.vector.tensor_copy(packed_b[:, hs, :], packed[:, hs, :])
                for h in range(2 * i, 2 * i + 2):
                    nc.sync.dma_start(
                        rdrow[0:1, h, :].rearrange("o (p j) -> o p j", j=JP),
                        packed_b[:, h, :],
                    )

            def emit_norm(h):
                p, hh = h // 2, h % 2
                for off, n in _tiles(NQ, 512):
                    ps = psp.tile([P, 512], f32, tag="ps", name="ps")
                    nc.tensor.matmul(
                        ps[0:64, 0:n],
                        lhsT=ones_sb[0:1, 0:64],
                        rhs=rdrow[0:1, h, off : off + n],
                        start=True,
                        stop=True,
                    )
                    av = AVT[p][hh * 64 : hh * 64 + 64, off : off + n]
                    nc.vector.tensor_mul(av, av, ps[0:64, 0:n])

            def emit_out(sci, m):
                osb = outp.tile([P, D], bf16, tag="osb", name="osb")
                for mh in range(2):
                    ps = psp.tile([P, 512], f32, tag="ps", name="ps")
                    for p in range(2):
                        nc.tensor.matmul(
                            ps[0:m, :],
                            lhsT=AVT[p][:, sci * P : sci * P + m],
                            rhs=wo_sb[:, p, mh * 512 : (mh + 1) * 512],
                            start=(p == 0),
                            stop=(p == 1),
                        )
                    nc.vector.tensor_copy(osb[0:m, mh * 512 : (mh + 1) * 512], ps[0:m, :])
                nc.sync.dma_start(out[sci * P : sci * P + m, :], osb[0:m, :])

            # ---- software-pipelined emission ----
            emit_maskbc()
            emit_qproj(0)
            emit_kproj(0, 0)
            emit_qproj(1)
            emit_kproj(1, 0)
            for tcn in range(8):
                emit_scores_tcn(0, 0, tcn)
                emit_vproj(0, tcn)
            emit_uav(0, 0, 0)
            emit_kproj(0, 1)
            emit_uav(0, 0, 1)
            emit_kproj(1, 1)
            for tcn in range(8):
                emit_scores_tcn(1, 0, tcn)
                emit_vproj(1, tcn)
            wo_sb = persist.tile([P, 2, D], bf16, tag="wo")
            nc.sync.dma_start(wo_sb[:], wo.rearrange("(c p) m -> p c m", p=P))
            emit_uav(1, 0, 2)
            emit_uav(1, 0, 3)
            for tcn in range(8):
                emit_scores_tcn(0, 1, tcn)
            emit_uav(0, 1, 0)
            emit_uav(0, 1, 1)
            emit_pack(0)
            emit_pack(1)
            for tcn in range(8):
                emit_scores_tcn(1, 1, tcn)
            emit_recip(0)
            emit_uav(1, 1, 2)
            emit_uav(1, 1, 3)
            emit_pack(2)
            emit_pack(3)
            emit_norm(0)
            emit_norm(1)
            emit_recip(1)
            emit_norm(2)
            emit_norm(3)
            nsc = _tiles(NQ, P)
            for sci, (off, m) in enumerate(nsc):
                emit_out(sci, m)

    _split_multiwait(nc)
    return nc


def _split_multiwait(nc):
    """This container's walrus rejects >1 sync wait on CTRL-class
    instructions (Tile's exit Drain carries one per outstanding proc).
    Hoist all but the last wait onto preceding same-engine NoOps."""
    import concourse.mybir as mybir

    for f in nc.m.functions:
        for bb in f.blocks:
            insts = list(bb.instructions)
            res, changed = [], False
            for inst in insts:
                si = inst.sync_info
                waits = list(si.on_wait) if si is not None else []
                if len(waits) > 1:
                    for w in waits[:-1]:
                        res.append(
                            mybir.InstNoOp(
                                name=nc.get_next_instruction_name(),
                                sync_info=mybir.SyncInfo(on_wait=[w], on_update=[]),
                                bass_nofuse=True,
                                engine=inst.engine,
                            )
                        )
                    inst.sync_info = mybir.SyncInfo(
                        on_wait=[waits[-1]], on_update=list(si.on_update)
                    )
                    changed = True
                res.append(inst)
            if changed:
                bb.instructions = res


def _plan(src_batch_lens):
    lens = [int(x) for x in np.asarray(src_batch_lens).reshape(-1)]
    need = [min(l, S) + 1 for l in lens]  # valid queries + 1 uniform slot
    order = sorted(range(B), key=lambda b: -need[b])
    pairs = [(order[0], order[3]), (order[1], order[2])]

    def r32(x):
        return min(S, ((x + 31) // 32) * 32)

    NA = r32(max(need[pairs[0][0]], need[pairs[1][0]]))
    NB = r32(max(need[pairs[0][1]], need[pairs[1][1]]))
    return lens, pairs, NA, NB


def _shard_inputs(x_Q, x_K, x_V, src_batch_lens, Wq, bq, Wk, bk, Wv, bv, Wo, bo):
    bf = ml_dtypes.bfloat16
    f32 = np.float32
    lens, pairs, NA, NB = _plan(src_batch_lens)
    NQ = NA + NB

    wq_all = (np.asarray(Wq, f32).transpose(1, 0, 2).reshape(D, H * DH) * SCALE).astype(bf)
    wk_all = np.asarray(Wk, f32).transpose(1, 0, 2).reshape(D, H * DH).astype(bf)
    wv_all = np.asarray(Wv, f32).transpose(1, 0, 2).reshape(D, H * DH).astype(bf)
    bq_all = (np.asarray(bq, f32).reshape(1, H * DH) * SCALE).astype(f32)
    bk_all = np.asarray(bk, f32).reshape(1, H * DH).astype(f32)
    bv_all = np.asarray(bv, f32).reshape(1, H * DH).astype(bf)
    wo_bf = np.asarray(Wo, f32).astype(bf)

    pair_data = []
    for bA, bB in pairs:
        xq = np.zeros((D, NQ), f32)
        m = np.zeros((1, NQ), f32)
        xk = np.empty((D, 2, S), f32)
        xv = np.empty((D, 2, S), f32)
        for slot, (b, off, nn) in enumerate(((bA, 0, NA), (bB, NA, NB))):
            ln = lens[b]
            xq[:, off : off + ln] = np.asarray(x_Q[b], f32).T[:, :ln]
            m[0, off : off + ln] = 1.0
            xk[:, slot, :] = np.asarray(x_K[b], f32).T
            xv[:, slot, :] = np.asarray(x_V[b], f32).T
        pair_data.append(
            (
                np.ascontiguousarray(xq).astype(bf),
                m.astype(bf),
                np.ascontiguousarray(xk).astype(bf),
                np.ascontiguousarray(xv).astype(bf),
            )
        )

    in_maps = []
    for c in range(8):
        p, hq = c // 4, c % 4
        hs = slice(hq * 256, (hq + 1) * 256)
        xqp, mp, xkp, xvp = pair_data[p]
        in_maps.append(
            {
                "xq": xqp,
                "xk": xkp,
                "xv": xvp,
                "wq": np.ascontiguousarray(wq_all[:, hs]),
                "wk": np.ascontiguousarray(wk_all[:, hs]),
                "wv": np.ascontiguousarray(wv_all[:, hs]),
                "wo": np.ascontiguousarray(wo_bf[hs, :]),
                "bq": np.ascontiguousarray(bq_all[:, hs]),
                "bk": np.ascontiguousarray(bk_all[:, hs]),
                "bv": np.ascontiguousarray(bv_all[:, hs]),
                "mask": mp,
            }
        )
    return in_maps


def kernel(**inputs):
    global _CACHED
    from concourse.bass_utils import run_bass_kernel_spmd

    lens, pairs, NA, NB = _plan(inputs["src_batch_lens"])
    key = (NA, NB)
    if key not in _CACHE:
        _CACHE[key] = _build(NA, NB)
    _CACHED = _CACHE[key]

    in_maps = _shard_inputs(**inputs)
    res = run_bass_kernel_spmd(_CACHED, in_maps, core_ids=list(range(8)))
    bo = np.asarray(inputs["bo"], np.float32)
    out = np.empty((B, S, D), np.float32)
    for p, (bA, bB) in enumerate(pairs):
        acc = np.zeros((NA + NB, D), np.float32)
        for hq in range(4):
            acc += np.asarray(res.results[4 * p + hq]["out"], np.float32)
        for b, off in ((bA, 0), (bB, NA)):
            ln = lens[b]
            out[b, :ln] = acc[off : off + ln]
            out[b, ln:] = acc[off + ln]
        out[bA] += bo[None, :]
        out[bB] += bo[None, :]
    return out


# revision 7
# speedup vs baseline: 1.3368x; 1.0885x over previous
"""Multi-head attention on 8 TRN2 NeuronCores.

Sharding: core c -> (batch-pair p = c//4, head-quarter q = c%4); each core
computes 4 heads x 2 batches. Queries are PACKED on the host: only the
first len_b valid query columns plus one zero column (whose softmax row
is uniform -> reproduces the reference's masked rows) are shipped, padded
to a unified (NA, NB) slot plan shared by both pairs; the host scatters
and broadcasts rows back afterwards. The program is compiled per (NA, NB)
at runtime, so any src_batch_lens values are handled exactly.

All-bf16 data path (fp8 anywhere adds ~2-3% error and busts the 2e-2
budget: per-key-independent noise on probs/V/AV survives softmax
averaging at full strength). Two exact algebraic removals instead:
  - bk is dropped entirely: Q . bk is constant across keys for a given
    query, and softmax is invariant to common-mode score shifts.
  - bv is folded into bo on the host (bo' = bo + bv_flat @ Wo), since
    sum_t softmax = 1 makes the bv term head-independent downstream.

Per-core layout: transposed attention (Q^T/K^T with head-dim on
partitions; scores^T per head with the two heads of a pair issued to
opposite 64-row PE row-groups so they execute concurrently; V natural
with a ones column carrying the softmax denominator through the AV^T
matmul; reciprocal via DMA-repack to 128 partitions; K=1 broadcast
matmuls, col-group-paired, for normalization; output projection against
this quarter's 256 rows of Wo). Host sums the 4 quarter-partials.

SBUF trick: batch-B keys are DMAed into the xq tile after the Q
projection has consumed it (Tile inserts the WAR sync automatically).
"""

import sys

sys.path.insert(0, "/opt/trn_rl_repo")

import numpy as np
import ml_dtypes

B, S, D, H, DH = 4, 1024, 1024, 16, 64
P = 128
SCALE = 1.0 / 8.0  # 1/sqrt(DH), folded into wq/bq on host

_CACHED = None  # last-built program (test.py compatibility)
_CACHE = {}


def _tiles(total, step):
    out = []
    off = 0
    while off < total:
        n = min(step, total - off)
        out.append((off, n))
        off += n
    return out


def _build(NA, NB):
    import concourse.bass as bass
    import concourse.mybir as mybir
    from concourse.tile import TileContext

    bf16 = mybir.dt.bfloat16
    f32 = mybir.dt.float32
    Exp = mybir.ActivationFunctionType.Exp

    NQ = NA + NB
    NQP = ((NQ + P - 1) // P) * P
    JP = NQP // P  # packed denominator columns per partition
    XQW = max(NQ, S)  # xq tile width (reused as batch-B key buffer)

    nc = bass.Bass()
    xq = nc.dram_tensor("xq", [D, NQ], bf16, kind="ExternalInput")
    xk = nc.dram_tensor("xk", [D, 2, S], bf16, kind="ExternalInput")
    xv = nc.dram_tensor("xv", [D, 2, S], bf16, kind="ExternalInput")
    wq = nc.dram_tensor("wq", [D, 256], bf16, kind="ExternalInput")  # pre-scaled
    wk = nc.dram_tensor("wk", [D, 256], bf16, kind="ExternalInput")
    wv = nc.dram_tensor("wv", [D, 256], bf16, kind="ExternalInput")
    wo = nc.dram_tensor("wo", [256, D], bf16, kind="ExternalInput")
    bqc = nc.dram_tensor("bq", [1, 256], f32, kind="ExternalInput")  # pre-scaled
    mask = nc.dram_tensor("mask", [1, NQ], bf16, kind="ExternalInput")
    out = nc.dram_tensor("out", [NQ, D], bf16, kind="ExternalOutput")

    QOFF = (0, NA)  # query-column offset per batch slot
    NB_ = (NA, NB)

    with TileContext(nc) as tc:
        with (
            tc.tile_pool(name="persist", bufs=1) as persist,
            tc.tile_pool(name="expa", bufs=2) as expa,
            tc.tile_pool(name="expb", bufs=1) as expb,
            tc.tile_pool(name="outp", bufs=3) as outp,
            tc.tile_pool(name="ps", bufs=4, space="PSUM") as psp,
            tc.tile_pool(name="sc", bufs=2, space="PSUM") as scp,
        ):
            # ---- small constants ----
            mask_sb = persist.tile([1, NQ], bf16, tag="mask")
            nc.sync.dma_start(mask_sb[:], mask[:])
            ones_sb = persist.tile([1, 512], bf16, tag="ones")
            nc.vector.memset(ones_sb[:], 1.0)
            bqc_sb = persist.tile([P, 2], f32, tag="bqc")
            nc.sync.dma_start(bqc_sb[:], bqc.rearrange("o (c p) -> p c o", p=P)[:, :, 0])
            mask_bc = persist.tile([P, NQ], bf16, tag="mask_bc")

            # ---- big inputs, chunked by d-chunk so matmuls start early ----
            xq_sb = persist.tile([P, 8, XQW], bf16, tag="xq")
            xk_sb = persist.tile([P, 8, S], bf16, tag="xk")  # batch A keys
            xv_sb = persist.tile([P, 8, 2 * S], bf16, tag="xv")
            wq_sb = persist.tile([P, 8, 256], bf16, tag="wq")
            wk_sb = persist.tile([P, 8, 256], bf16, tag="wk")
            wv_sb = persist.tile([P, 8, 256], bf16, tag="wv")
            wo_sb = persist.tile([P, 2, D], bf16, tag="wo")
            xq_r = xq.rearrange("(c p) s -> p c s", p=P)
            xk_r = xk.rearrange("(c p) b s -> p c b s", p=P)
            xv_r = xv.rearrange("(c p) b s -> p c b s", p=P)
            wq_r = wq.rearrange("(c p) m -> p c m", p=P)
            wk_r = wk.rearrange("(c p) m -> p c m", p=P)
            wv_r = wv.rearrange("(c p) m -> p c m", p=P)
            xv_v = xv_sb[:].rearrange("p c (b s) -> p c b s", b=2)
            for dc in range(8):
                eng = nc.sync if dc % 2 == 0 else nc.scalar
                eng.dma_start(wq_sb[:, dc, :], wq_r[:, dc, :])
                eng.dma_start(xq_sb[:, dc, 0:NQ], xq_r[:, dc, :])
                nc.gpsimd.dma_start(wk_sb[:, dc, :], wk_r[:, dc, :])
                nc.gpsimd.dma_start(xk_sb[:, dc, :], xk_r[:, dc, 0, :])
            for dc in range(8):
                nc.gpsimd.dma_start(wv_sb[:, dc, :], wv_r[:, dc, :])
                eng = nc.sync if dc % 2 == 0 else nc.gpsimd
                eng.dma_start(xv_v[:, dc, 0, :], xv_r[:, dc, 0, :])
                eng.dma_start(xv_v[:, dc, 1, :], xv_r[:, dc, 1, :])
            nc.gpsimd.dma_start(wo_sb[:], wo.rearrange("(c p) m -> p c m", p=P))

            QT = [persist.tile([P, NQ], bf16, tag=f"qt{p}", name=f"qt{p}") for p in range(2)]
            KT = [persist.tile([P, 2, S], bf16, tag=f"kt{p}", name=f"kt{p}") for p in range(2)]
            AVT = [persist.tile([P, NQ], bf16, tag=f"avt{p}", name=f"avt{p}") for p in range(2)]
            vaug = persist.tile([P, 8, 2, 260], bf16, tag="vaug")
            nc.vector.memset(vaug[:], 1.0)

            exps = {}  # (pair, b) -> bf16 prob tile [P, 8, 2, N_b]
            stages = persist.tile([1, 4, NQP], bf16, tag="stages")
            nc.vector.memset(stages[:], 1.0)
            packed = persist.tile([P, 4, JP], bf16, tag="packed")
            recipf = persist.tile([P, 4, JP], f32, tag="recipf")
            packed_b = persist.tile([P, 4, JP], bf16, tag="packedb")
            rdrow = persist.tile([1, 4, NQP], bf16, tag="rdrow")

            def emit_maskbc():
                for off, n in _tiles(NQ, 512):
                    ps = psp.tile([P, 512], f32, tag="ps", name="ps")
                    nc.tensor.matmul(
                        ps[:, 0:n],
                        lhsT=ones_sb[0:1, 0:P],
                        rhs=mask_sb[0:1, off : off + n],
                        start=True,
                        stop=True,
                    )
                    nc.vector.tensor_copy(mask_bc[:, off : off + n], ps[:, 0:n])

            def emit_qkproj(p, w_sb, x_ap, width, dst, epilogue):
                # dc-outer / tile-inner with parallel psums: each LDWEIGHTS
                # serves all column tiles of the row-chunk.
                tl = _tiles(width, 512)
                pss = [psp.tile([P, 512], f32, tag="ps", name="ps") for _ in tl]
                for dc in range(8):
                    for ti, (off, n) in enumerate(tl):
                        nc.tensor.matmul(
                            pss[ti][:, 0:n],
                            lhsT=w_sb[:, dc, p * P : (p + 1) * P],
                            rhs=x_ap[:, dc, off : off + n],
                            start=(dc == 0),
                            stop=(dc == 7),
                        )
                for ti, (off, n) in enumerate(tl):
                    epilogue(dst, off, n, pss[ti])

            def q_epi(p, off, n, ps):
                nc.vector.scalar_tensor_tensor(
                    QT[p][:, off : off + n],
                    ps[:, 0:n],
                    bqc_sb[:, p : p + 1],
                    mask_bc[:, off : off + n],
                    mybir.AluOpType.add,
                    mybir.AluOpType.mult,
                )

            def emit_qproj(p):
                emit_qkproj(
                    p, wq_sb, xq_sb[:], NQ, p, lambda p_, off, n, ps: q_epi(p_, off, n, ps)
                )

            def emit_kproj(p, b, x_ap):
                def k_epi(_, off, n, ps):
                    nc.vector.tensor_copy(KT[p][:, b, off : off + n], ps[:, 0:n])

                emit_qkproj(p, wk_sb, x_ap, S, None, k_epi)

            def emit_vproj(b, tcn):
                ps = psp.tile([P, 512], f32, tag="ps", name="ps")
                for dc in range(8):
                    nc.tensor.matmul(
                        ps[:, 0:256],
                        lhsT=xv_v[:, dc, b, tcn * P : (tcn + 1) * P],
                        rhs=wv_sb[:, dc, 0:256],
                        start=(dc == 0),
                        stop=(dc == 7),
                    )
                nc.vector.tensor_copy(
                    vaug[:, tcn, b, :]
                    .rearrange("p (h x) -> p h x", x=65)[:, :, 0:64],
                    ps[:, 0:256].rearrange("p (h v) -> p h v", v=64),
                )

            def emit_scores_tcn(p, b, tcn):
                if (p, b) not in exps:
                    pool = expa if b == 0 else expb
                    exps[(p, b)] = pool.tile(
                        [P, 8, 2, NB_[b]], bf16, tag=f"exps{b}", name=f"exps{b}"
                    )
                ex = exps[(p, b)]
                qo = QOFF[b]
                for off, n in _tiles(NB_[b], 512):
                    sc = scp.tile([P, 2, 512], f32, tag="sc", name="sc")
                    for hh in range(2):
                        nc.tensor.matmul(
                            sc[:, hh, 0:n],
                            lhsT=KT[p][hh * 64 : hh * 64 + 64, b, tcn * P : (tcn + 1) * P],
                            rhs=QT[p][hh * 64 : hh * 64 + 64, qo + off : qo + off + n],
                            start=True,
                            stop=True,
                        )
                    nc.scalar.activation(
                        ex[:, tcn, :, off : off + n], sc[:, :, 0:n], Exp
                    )

            def emit_uav(p, b, h, only_tile=None):
                hh = h % 2
                ex = exps[(p, b)]
                qo = QOFF[b]
                for ti, (off, n) in enumerate(_tiles(NB_[b], 512)):
                    if only_tile is not None and ti != only_tile:
                        continue
                    ps = psp.tile([P, 512], f32, tag="ps", name="ps")
                    for tcn in range(8):
                        nc.tensor.matmul(
                            ps[0:65, 0:n],
                            lhsT=vaug[:, tcn, b, h * 65 : h * 65 + 65],
                            rhs=ex[:, tcn, hh, off : off + n],
                            start=(tcn == 0),
                            stop=(tcn == 7),
                        )
                    nc.vector.tensor_copy(
                        AVT[p][hh * 64 : hh * 64 + 64, qo + off : qo + off + n],
                        ps[0:64, 0:n],
                    )
                    nc.vector.tensor_copy(
                        stages[0:1, h, qo + off : qo + off + n], ps[64:65, 0:n]
                    )

            def emit_pack(h):
                nc.sync.dma_start(
                    packed[:, h, :],
                    stages[0:1, h, :].rearrange("o (p j) -> o p j", j=JP),
                )

            def emit_recip(i):
                hs = slice(2 * i, 2 * i + 2)
                nc.vector.reciprocal(recipf[:, hs, :], packed[:, hs, :])
                nc.vector.tensor_copy(packed_b[:, hs, :], recipf[:, hs, :])
                for h in range(2 * i, 2 * i + 2):
                    nc.sync.dma_start(
                        rdrow[0:1, h, :].rearrange("o (p j) -> o p j", j=JP),
                        packed_b[:, h, :],
                    )

            def emit_norm(p, off, n):
                # both heads of the pair in one psum via opposite col-groups,
                # then a single full-height multiply.
                ps = psp.tile([P, 512], f32, tag="ps", name="ps")
                nc.tensor.matmul(
                    ps[0:64, 0:n],
                    lhsT=ones_sb[0:1, 0:64],
                    rhs=rdrow[0:1, 2 * p, off : off + n],
                    start=True,
                    stop=True,
                )
                nc.tensor.matmul(
                    ps[64:128, 0:n],
                    lhsT=ones_sb[0:1, 0:64],
                    rhs=rdrow[0:1, 2 * p + 1, off : off + n],
                    start=True,
                    stop=True,
                )
                av = AVT[p][:, off : off + n]
                nc.vector.tensor_mul(av, av, ps[:, 0:n])

            def emit_out(sci, m):
                osb = outp.tile([P, D], bf16, tag="osb", name="osb")
                pss = [psp.tile([P, 512], f32, tag="ps", name="ps") for _ in range(2)]
                for p in range(2):
                    for mh in range(2):
                        nc.tensor.matmul(
                            pss[mh][0:m, :],
                            lhsT=AVT[p][:, sci * P : sci * P + m],
                            rhs=wo_sb[:, p, mh * 512 : (mh + 1) * 512],
                            start=(p == 0),
                            stop=(p == 1),
                        )
                nc.vector.tensor_copy(osb[0:m, 0:512], pss[0][0:m, :])
                nc.scalar.activation(
                    osb[0:m, 512:1024],
                    pss[1][0:m, :],
                    mybir.ActivationFunctionType.Copy,
                )
                nc.sync.dma_start(out[sci * P : sci * P + m, :], osb[0:m, :])

            # ---- software-pipelined emission ----
            emit_maskbc()
            emit_qproj(0)
            emit_kproj(0, 0, xk_sb[:])
            emit_qproj(1)
            emit_kproj(1, 0, xk_sb[:])
            # batch-B keys into the (now consumed) xq tile
            for dc in range(8):
                eng = nc.sync if dc % 2 == 0 else nc.scalar
                eng.dma_start(xq_sb[:, dc, 0:S], xk_r[:, dc, 1, :])

            for tcn in range(8):
                emit_scores_tcn(0, 0, tcn)
                emit_vproj(0, tcn)
            for tcn in range(8):
                emit_scores_tcn(1, 0, tcn)
                emit_vproj(1, tcn)
            emit_uav(0, 0, 0)
            emit_kproj(0, 1, xq_sb[:])
            emit_uav(0, 0, 1)
            emit_kproj(1, 1, xq_sb[:])
            # batch-B attention for pair 0 first (expb has a single buffer);
            # pair-1 batch-A AV matmuls keep the PE fed under the exp shadow.
            for tcn in range(8):
                emit_scores_tcn(0, 1, tcn)
                if tcn == 2:
                    emit_uav(1, 0, 2)
                elif tcn == 5:
                    emit_uav(1, 0, 3)
            emit_uav(0, 1, 0)
            emit_uav(0, 1, 1)
            emit_pack(0)
            emit_pack(1)
            emit_recip(0)
            for tcn in range(8):
                emit_scores_tcn(1, 1, tcn)
            for off, n in _tiles(NQ, 512):
                emit_norm(0, off, n)
            emit_uav(1, 1, 2)
            emit_uav(1, 1, 3)
            emit_pack(2)
            emit_pack(3)
            emit_recip(1)
            for off, n in _tiles(NQ, 512):
                emit_norm(1, off, n)
            for sci, (off, m) in enumerate(_tiles(NQ, P)):
                emit_out(sci, m)

    _split_multiwait(nc)
    return nc


def _split_multiwait(nc):
    """This container's walrus rejects >1 sync wait on CTRL-class
    instructions (Tile's exit Drain carries one per outstanding proc).
    Hoist all but the last wait onto preceding same-engine NoOps."""
    import concourse.mybir as mybir

    for f in nc.m.functions:
        for bb in f.blocks:
            insts = list(bb.instructions)
            res, changed = [], False
            for inst in insts:
                si = inst.sync_info
                waits = list(si.on_wait) if si is not None else []
                if len(waits) > 1:
                    for w in waits[:-1]:
                        res.append(
                            mybir.InstNoOp(
                                name=nc.get_next_instruction_name(),
                                sync_info=mybir.SyncInfo(on_wait=[w], on_update=[]),
                                bass_nofuse=True,
                                engine=inst.engine,
                            )
                        )
                    inst.sync_info = mybir.SyncInfo(
                        on_wait=[waits[-1]], on_update=list(si.on_update)
                    )
                    changed = True
                res.append(inst)
            if changed:
                bb.instructions = res


def _plan(src_batch_lens):
    lens = [int(x) for x in np.asarray(src_batch_lens).reshape(-1)]
    need = [min(l, S) + 1 for l in lens]  # valid queries + 1 uniform slot
    order = sorted(range(B), key=lambda b: -need[b])
    pairs = [(order[0], order[3]), (order[1], order[2])]

    def r32(x):
        return min(S, ((x + 31) // 32) * 32)

    NA = r32(max(need[pairs[0][0]], need[pairs[1][0]]))
    NB = r32(max(need[pairs[0][1]], need[pairs[1][1]]))
    return lens, pairs, NA, NB


def _shard_inputs(x_Q, x_K, x_V, src_batch_lens, Wq, bq, Wk, bk, Wv, bv, Wo, bo):
    bf = ml_dtypes.bfloat16
    f32 = np.float32
    lens, pairs, NA, NB = _plan(src_batch_lens)
    NQ = NA + NB

    wq_all = (np.asarray(Wq, f32).transpose(1, 0, 2).reshape(D, H * DH) * SCALE).astype(bf)
    wk_all = np.asarray(Wk, f32).transpose(1, 0, 2).reshape(D, H * DH).astype(bf)
    wv_all = np.asarray(Wv, f32).transpose(1, 0, 2).reshape(D, H * DH).astype(bf)
    bq_all = (np.asarray(bq, f32).reshape(1, H * DH) * SCALE).astype(f32)
    wo_bf = np.asarray(Wo, f32).astype(bf)

    pair_data = []
    for bA, bB in pairs:
        xq = np.zeros((D, NQ), f32)
        m = np.zeros((1, NQ), f32)
        xk = np.empty((D, 2, S), f32)
        xv = np.empty((D, 2, S), f32)
        for slot, (b, off) in enumerate(((bA, 0), (bB, NA))):
            ln = lens[b]
            xq[:, off : off + ln] = np.asarray(x_Q[b], f32).T[:, :ln]
            m[0, off : off + ln] = 1.0
            xk[:, slot, :] = np.asarray(x_K[b], f32).T
            xv[:, slot, :] = np.asarray(x_V[b], f32).T
        pair_data.append(
            (
                np.ascontiguousarray(xq).astype(bf),
                m.astype(bf),
                np.ascontiguousarray(xk).astype(bf),
                np.ascontiguousarray(xv).astype(bf),
            )
        )

    in_maps = []
    for c in range(8):
        p, hq = c // 4, c % 4
        hs = slice(hq * 256, (hq + 1) * 256)
        xqp, mp, xkp, xvp = pair_data[p]
        in_maps.append(
            {
                "xq": xqp,
                "xk": xkp,
                "xv": xvp,
                "wq": np.ascontiguousarray(wq_all[:, hs]),
                "wk": np.ascontiguousarray(wk_all[:, hs]),
                "wv": np.ascontiguousarray(wv_all[:, hs]),
                "wo": np.ascontiguousarray(wo_bf[hs, :]),
                "bq": np.ascontiguousarray(bq_all[:, hs]),
                "mask": mp,
            }
        )
    return in_maps


def kernel(**inputs):
    global _CACHED
    from concourse.bass_utils import run_bass_kernel_spmd

    lens, pairs, NA, NB = _plan(inputs["src_batch_lens"])
    key = (NA, NB)
    if key not in _CACHE:
        _CACHE[key] = _build(NA, NB)
    _CACHED = _CACHE[key]

    in_maps = _shard_inputs(**inputs)
    res = run_bass_kernel_spmd(_CACHED, in_maps, core_ids=list(range(8)))
    # bv folds into an effective output bias: sum_h bv_h @ Wo_h + bo
    bo_eff = (
        np.asarray(bo := inputs["bo"], np.float32)
        + np.asarray(inputs["bv"], np.float32).reshape(-1)
        @ np.asarray(inputs["Wo"], np.float32)
    )
    out = np.empty((B, S, D), np.float32)
    for p, (bA, bB) in enumerate(pairs):
        acc = np.zeros((NA + NB, D), np.float32)
        for hq in range(4):
            acc += np.asarray(res.results[4 * p + hq]["out"], np.float32)
        for b, off in ((bA, 0), (bB, NA)):
            ln = lens[b]
            out[b, :ln] = acc[off : off + ln]
            out[b, ln:] = acc[off + ln]
            out[b] += bo_eff[None, :]
    return out


# revision 15
# speedup vs baseline: 1.3532x; 1.0123x over previous
"""Multi-head attention on 8 TRN2 NeuronCores.

Sharding: core c -> (batch-pair p = c//4, head-quarter q = c%4); each core
computes 4 heads x 2 batches. Queries are PACKED on the host: only the
first len_b valid query columns plus one zero column (whose softmax row
is uniform -> reproduces the reference's masked rows) are shipped, padded
to a unified (NA, NB) slot plan shared by both pairs; the host scatters
and broadcasts rows back afterwards. The program is compiled per (NA, NB)
at runtime, so any src_batch_lens values are handled exactly.

All-bf16 data path (fp8 anywhere adds ~2-3% error and busts the 2e-2
budget: per-key-independent noise on probs/V/AV survives softmax
averaging at full strength). Two exact algebraic removals instead:
  - bk is dropped entirely: Q . bk is constant across keys for a given
    query, and softmax is invariant to common-mode score shifts.
  - bv is folded into bo on the host (bo' = bo + bv_flat @ Wo), since
    sum_t softmax = 1 makes the bv term head-independent downstream.

Per-core layout: transposed attention (Q^T/K^T with head-dim on
partitions; scores^T per head with the two heads of a pair issued to
opposite 64-row PE row-groups so they execute concurrently; V natural
with a ones column carrying the softmax denominator through the AV^T
matmul; reciprocal via DMA-repack to 128 partitions; K=1 broadcast
matmuls, col-group-paired, for normalization; output projection against
this quarter's 256 rows of Wo). Host sums the 4 quarter-partials.

SBUF trick: batch-B keys are DMAed into the xq tile after the Q
projection has consumed it (Tile inserts the WAR sync automatically).
"""

import sys

sys.path.insert(0, "/opt/trn_rl_repo")

import numpy as np
import ml_dtypes

B, S, D, H, DH = 4, 1024, 1024, 16, 64
P = 128
SCALE = 1.0 / 8.0  # 1/sqrt(DH), folded into wq/bq on host

_CACHED = None  # last-built program (test.py compatibility)
_CACHE = {}


def _tiles(total, step):
    out = []
    off = 0
    while off < total:
        n = min(step, total - off)
        out.append((off, n))
        off += n
    return out


def _build(NA, NB):
    import concourse.bass as bass
    import concourse.mybir as mybir
    from concourse.tile import TileContext

    bf16 = mybir.dt.bfloat16
    f32 = mybir.dt.float32
    Exp = mybir.ActivationFunctionType.Exp

    NQ = NA + NB
    JA, JB = NA // 64, NB // 64  # per-region denominator repack columns
    XQW = max(NQ, S)  # xq tile width (reused as batch-B key buffer)

    nc = bass.Bass()
    xq = nc.dram_tensor("xq", [D, NQ], bf16, kind="ExternalInput")
    xk = nc.dram_tensor("xk", [D, 2, S], bf16, kind="ExternalInput")
    xv = nc.dram_tensor("xv", [D, 2, S], bf16, kind="ExternalInput")
    wq = nc.dram_tensor("wq", [D, 256], bf16, kind="ExternalInput")  # pre-scaled
    wk = nc.dram_tensor("wk", [D, 256], bf16, kind="ExternalInput")
    wv = nc.dram_tensor("wv", [D, 256], bf16, kind="ExternalInput")
    wo = nc.dram_tensor("wo", [256, D], bf16, kind="ExternalInput")
    bqc = nc.dram_tensor("bq", [1, 256], f32, kind="ExternalInput")  # pre-scaled
    mask = nc.dram_tensor("mask", [1, NQ], bf16, kind="ExternalInput")
    out = nc.dram_tensor("out", [NQ, D], bf16, kind="ExternalOutput")

    QOFF = (0, NA)  # query-column offset per batch slot
    NB_ = (NA, NB)

    with TileContext(nc) as tc:
        with (
            tc.tile_pool(name="persist", bufs=1) as persist,
            tc.tile_pool(name="expa", bufs=2) as expa,
            tc.tile_pool(name="expb", bufs=1) as expb,
            tc.tile_pool(name="outp", bufs=3) as outp,
            tc.tile_pool(name="ps", bufs=4, space="PSUM") as psp,
            tc.tile_pool(name="sc", bufs=2, space="PSUM") as scp,
        ):
            # ---- small constants ----
            mask_sb = persist.tile([1, NQ], bf16, tag="mask")
            nc.sync.dma_start(mask_sb[:], mask[:])
            ones_sb = persist.tile([1, 512], bf16, tag="ones")
            nc.vector.memset(ones_sb[:], 1.0)
            bqc_sb = persist.tile([P, 2], f32, tag="bqc")
            nc.sync.dma_start(bqc_sb[:], bqc.rearrange("o (c p) -> p c o", p=P)[:, :, 0])
            mask_bc = persist.tile([P, NQ], bf16, tag="mask_bc")

            # ---- big inputs, chunked by d-chunk so matmuls start early ----
            xq_sb = persist.tile([P, 8, XQW], bf16, tag="xq")
            xk_sb = persist.tile([P, 8, S], bf16, tag="xk")  # batch A keys
            xv_sb = persist.tile([P, 8, 2 * S], bf16, tag="xv")
            wq_sb = persist.tile([P, 8, 256], bf16, tag="wq")
            wk_sb = persist.tile([P, 8, 256], bf16, tag="wk")
            wv_sb = persist.tile([P, 8, 256], bf16, tag="wv")
            wo_sb = persist.tile([P, 2, D], bf16, tag="wo")
            xq_r = xq.rearrange("(c p) s -> p c s", p=P)
            xk_r = xk.rearrange("(c p) b s -> p c b s", p=P)
            xv_r = xv.rearrange("(c p) b s -> p c b s", p=P)
            wq_r = wq.rearrange("(c p) m -> p c m", p=P)
            wk_r = wk.rearrange("(c p) m -> p c m", p=P)
            wv_r = wv.rearrange("(c p) m -> p c m", p=P)
            xv_v = xv_sb[:].rearrange("p c (b s) -> p c b s", b=2)
            # priority order: Q/K-proj inputs first, V inputs after
            for dc in range(8):
                eng = nc.sync if dc % 2 == 0 else nc.scalar
                eng.dma_start(wq_sb[:, dc, :], wq_r[:, dc, :])
                eng.dma_start(xq_sb[:, dc, 0:NQ], xq_r[:, dc, :])
                nc.gpsimd.dma_start(wk_sb[:, dc, :], wk_r[:, dc, :])
                nc.gpsimd.dma_start(xk_sb[:, dc, :], xk_r[:, dc, 0, :])
            for dc in range(8):
                nc.gpsimd.dma_start(wv_sb[:, dc, :], wv_r[:, dc, :])
                eng = nc.sync if dc % 2 == 0 else nc.gpsimd
                eng.dma_start(xv_v[:, dc, 0, :], xv_r[:, dc, 0, :])
                eng.dma_start(xv_v[:, dc, 1, :], xv_r[:, dc, 1, :])
            nc.scalar.dma_start(wo_sb[:], wo.rearrange("(c p) m -> p c m", p=P))

            QT = [persist.tile([P, NQ], bf16, tag=f"qt{p}", name=f"qt{p}") for p in range(2)]
            KT = [persist.tile([P, 2, S], bf16, tag=f"kt{p}", name=f"kt{p}") for p in range(2)]
            AVT = [persist.tile([P, NQ], bf16, tag=f"avt{p}", name=f"avt{p}") for p in range(2)]
            vaug = persist.tile([P, 8, 2, 260], bf16, tag="vaug")
            nc.vector.memset(vaug[:], 1.0)

            exps = {}  # (pair, b) -> bf16 prob tile [P, 8, 2, N_b]
            stages = persist.tile([1, 4, NQ], bf16, tag="stages")
            packed = persist.tile([64, 4, JA + JB], bf16, tag="packed")
            recipf = persist.tile([64, 4, JA + JB], f32, tag="recipf")
            packed_b = persist.tile([64, 4, JA + JB], bf16, tag="packedb")
            rdrow = persist.tile([1, 4, NQ], bf16, tag="rdrow")

            def emit_maskbc():
                for off, n in _tiles(NQ, 512):
                    ps = psp.tile([P, 512], f32, tag="ps", name="ps")
                    nc.tensor.matmul(
                        ps[:, 0:n],
                        lhsT=ones_sb[0:1, 0:P],
                        rhs=mask_sb[0:1, off : off + n],
                        start=True,
                        stop=True,
                    )
                    nc.vector.tensor_copy(mask_bc[:, off : off + n], ps[:, 0:n])

            def emit_qkproj(p, w_sb, x_ap, width, dst, epilogue):
                # dc-outer / tile-inner with parallel psums: each LDWEIGHTS
                # serves all column tiles of the row-chunk.
                tl = _tiles(width, 512)
                pss = [psp.tile([P, 512], f32, tag="ps", name="ps") for _ in tl]
                for dc in range(8):
                    for ti, (off, n) in enumerate(tl):
                        nc.tensor.matmul(
                            pss[ti][:, 0:n],
                            lhsT=w_sb[:, dc, p * P : (p + 1) * P],
                            rhs=x_ap[:, dc, off : off + n],
                            start=(dc == 0),
                            stop=(dc == 7),
                        )
                for ti, (off, n) in enumerate(tl):
                    epilogue(dst, off, n, pss[ti])

            def q_epi(p, off, n, ps):
                nc.vector.scalar_tensor_tensor(
                    QT[p][:, off : off + n],
                    ps[:, 0:n],
                    bqc_sb[:, p : p + 1],
                    mask_bc[:, off : off + n],
                    mybir.AluOpType.add,
                    mybir.AluOpType.mult,
                )

            def emit_qproj(p):
                emit_qkproj(
                    p, wq_sb, xq_sb[:], NQ, p, lambda p_, off, n, ps: q_epi(p_, off, n, ps)
                )

            def emit_kproj(p, b, x_ap):
                def k_epi(_, off, n, ps):
                    nc.vector.tensor_copy(KT[p][:, b, off : off + n], ps[:, 0:n])

                emit_qkproj(p, wk_sb, x_ap, S, None, k_epi)

            def emit_vproj(b, tcn):
                ps = psp.tile([P, 512], f32, tag="ps", name="ps")
                for dc in range(8):
                    nc.tensor.matmul(
                        ps[:, 0:256],
                        lhsT=xv_v[:, dc, b, tcn * P : (tcn + 1) * P],
                        rhs=wv_sb[:, dc, 0:256],
                        start=(dc == 0),
                        stop=(dc == 7),
                    )
                nc.vector.tensor_copy(
                    vaug[:, tcn, b, :]
                    .rearrange("p (h x) -> p h x", x=65)[:, :, 0:64],
                    ps[:, 0:256].rearrange("p (h v) -> p h v", v=64),
                )

            def emit_scores_tcn(p, b, tcn):
                if (p, b) not in exps:
                    pool = expa if b == 0 else expb
                    exps[(p, b)] = pool.tile(
                        [P, 8, 2, NB_[b]], bf16, tag=f"exps{b}", name=f"exps{b}"
                    )
                ex = exps[(p, b)]
                qo = QOFF[b]
                for off, n in _tiles(NB_[b], 512):
                    sc = scp.tile([P, 2, 512], f32, tag="sc", name="sc")
                    for hh in range(2):
                        nc.tensor.matmul(
                            sc[:, hh, 0:n],
                            lhsT=KT[p][hh * 64 : hh * 64 + 64, b, tcn * P : (tcn + 1) * P],
                            rhs=QT[p][hh * 64 : hh * 64 + 64, qo + off : qo + off + n],
                            start=True,
                            stop=True,
                        )
                    nc.scalar.activation(
                        ex[:, tcn, :, off : off + n], sc[:, :, 0:n], Exp
                    )

            def emit_uav(p, b, h, only_tile=None):
                hh = h % 2
                ex = exps[(p, b)]
                qo = QOFF[b]
                for ti, (off, n) in enumerate(_tiles(NB_[b], 512)):
                    if only_tile is not None and ti != only_tile:
                        continue
                    ps = psp.tile([P, 512], f32, tag="ps", name="ps")
                    for tcn in range(8):
                        nc.tensor.matmul(
                            ps[0:65, 0:n],
                            lhsT=vaug[:, tcn, b, h * 65 : h * 65 + 65],
                            rhs=ex[:, tcn, hh, off : off + n],
                            start=(tcn == 0),
                            stop=(tcn == 7),
                        )
                    nc.vector.tensor_copy(
                        AVT[p][hh * 64 : hh * 64 + 64, qo + off : qo + off + n],
                        ps[0:64, 0:n],
                    )
                    nc.vector.tensor_copy(
                        stages[0:1, h, qo + off : qo + off + n], ps[64:65, 0:n]
                    )

            # region r: 0 = batch-A query columns [0, NA), 1 = [NA, NQ)
            RJ = (JA, JB)

            def emit_pack(h, r):
                qo, jr = QOFF[r], RJ[r]
                jo = 0 if r == 0 else JA
                nc.sync.dma_start(
                    packed[:, h, jo : jo + jr],
                    stages[0:1, h, qo : qo + NB_[r]].rearrange(
                        "o (p j) -> o p j", j=jr
                    ),
                )

            def emit_recip(r):
                # reciprocal of all 4 heads' region-r denominators at once
                js = slice(0, JA) if r == 0 else slice(JA, JA + JB)
                nc.vector.reciprocal(recipf[:, :, js], packed[:, :, js])
                nc.vector.tensor_copy(packed_b[:, :, js], recipf[:, :, js])
                qo, jr = QOFF[r], RJ[r]
                for h in range(4):
                    nc.sync.dma_start(
                        rdrow[0:1, h, qo : qo + NB_[r]].rearrange(
                            "o (p j) -> o p j", j=jr
                        ),
                        packed_b[:, h, js],
                    )

            def emit_norm(p, off, n):
                # both heads of the pair in one psum via opposite col-groups,
                # then a single full-height multiply.
                ps = psp.tile([P, 512], f32, tag="ps", name="ps")
                nc.tensor.matmul(
                    ps[0:64, 0:n],
                    lhsT=ones_sb[0:1, 0:64],
                    rhs=rdrow[0:1, 2 * p, off : off + n],
                    start=True,
                    stop=True,
                )
                nc.tensor.matmul(
                    ps[64:128, 0:n],
                    lhsT=ones_sb[0:1, 0:64],
                    rhs=rdrow[0:1, 2 * p + 1, off : off + n],
                    start=True,
                    stop=True,
                )
                av = AVT[p][:, off : off + n]
                nc.vector.tensor_mul(av, av, ps[:, 0:n])

            def emit_out(off, m):
                osb = outp.tile([P, D], bf16, tag="osb", name="osb")
                pss = [psp.tile([P, 512], f32, tag="ps", name="ps") for _ in range(2)]
                for p in range(2):
                    for mh in range(2):
                        nc.tensor.matmul(
                            pss[mh][0:m, :],
                            lhsT=AVT[p][:, off : off + m],
                            rhs=wo_sb[:, p, mh * 512 : (mh + 1) * 512],
                            start=(p == 0),
                            stop=(p == 1),
                        )
                nc.vector.tensor_copy(osb[0:m, 0:512], pss[0][0:m, :])
                nc.scalar.activation(
                    osb[0:m, 512:1024],
                    pss[1][0:m, :],
                    mybir.ActivationFunctionType.Copy,
                )
                nc.sync.dma_start(out[off : off + m, :], osb[0:m, :])

            # ---- software-pipelined emission ----
            emit_maskbc()
            emit_qproj(0)
            emit_kproj(0, 0, xk_sb[:])
            emit_qproj(1)
            emit_kproj(1, 0, xk_sb[:])
            # batch-B keys into the (now consumed) xq tile
            for dc in range(8):
                eng = nc.sync if dc % 2 == 0 else nc.scalar
                eng.dma_start(xq_sb[:, dc, 0:S], xk_r[:, dc, 1, :])

            for tcn in range(8):
                emit_scores_tcn(0, 0, tcn)
                emit_vproj(0, tcn)
            for tcn in range(8):
                emit_scores_tcn(1, 0, tcn)
                emit_vproj(1, tcn)
            emit_uav(0, 0, 0)
            emit_kproj(0, 1, xq_sb[:])
            emit_uav(0, 0, 1)
            emit_kproj(1, 1, xq_sb[:])
            emit_uav(1, 0, 2)
            emit_uav(1, 0, 3)
            # A-region denominators are complete (they only involve batch-A
            # keys): reciprocal + normalization + the A-rows of the output
            # projection all run under the batch-B exp shadow below.
            for h in range(4):
                emit_pack(h, 0)
            emit_recip(0)
            norm_a = [(p, off, n) for p in range(2) for off, n in _tiles(NA, 512)]
            out_a = _tiles(NA, P)
            out_b = [(NA + off, m) for off, m in _tiles(NB, P)]
            na, oa = 0, 0
            for tcn in range(8):
                emit_scores_tcn(0, 1, tcn)
                while na < len(norm_a) and na <= tcn:
                    p_, off, n = norm_a[na]
                    emit_norm(p_, off, n)
                    na += 1
                if tcn >= 4 and oa < len(out_a):
                    off, m = out_a[oa]
                    emit_out(off, m)
                    oa += 1
            while na < len(norm_a):
                p_, off, n = norm_a[na]
                emit_norm(p_, off, n)
                na += 1
            emit_uav(0, 1, 0)
            emit_uav(0, 1, 1)
            emit_pack(0, 1)
            emit_pack(1, 1)
            for tcn in range(8):
                emit_scores_tcn(1, 1, tcn)
                if oa < len(out_a):
                    off, m = out_a[oa]
                    emit_out(off, m)
                    oa += 1
            emit_uav(1, 1, 2)
            emit_uav(1, 1, 3)
            while oa < len(out_a):
                off, m = out_a[oa]
                emit_out(off, m)
                oa += 1
            emit_pack(2, 1)
            emit_pack(3, 1)
            emit_recip(1)
            for off, n in _tiles(NB, 512):
                emit_norm(0, NA + off, n)
                emit_norm(1, NA + off, n)
            for off, m in out_b:
                emit_out(off, m)

    _split_multiwait(nc)
    return nc


def _split_multiwait(nc):
    """This container's walrus rejects >1 sync wait on CTRL-class
    instructions (Tile's exit Drain carries one per outstanding proc).
    Hoist all but the last wait onto preceding same-engine NoOps."""
    import concourse.mybir as mybir

    for f in nc.m.functions:
        for bb in f.blocks:
            insts = list(bb.instructions)
            res, changed = [], False
            for inst in insts:
                si = inst.sync_info
                waits = list(si.on_wait) if si is not None else []
                if len(waits) > 1:
                    for w in waits[:-1]:
                        res.append(
                            mybir.InstNoOp(
                                name=nc.get_next_instruction_name(),
                                sync_info=mybir.SyncInfo(on_wait=[w], on_update=[]),
                                bass_nofuse=True,
                                engine=inst.engine,
                            )
                        )
                    inst.sync_info = mybir.SyncInfo(
                        on_wait=[waits[-1]], on_update=list(si.on_update)
                    )
                    changed = True
                res.append(inst)
            if changed:
                bb.instructions = res


def _plan(src_batch_lens):
    lens = [int(x) for x in np.asarray(src_batch_lens).reshape(-1)]
    need = [min(l, S) + 1 for l in lens]  # valid queries + 1 uniform slot
    order = sorted(range(B), key=lambda b: -need[b])
    pairs = [(order[0], order[3]), (order[1], order[2])]

    def r64(x):
        return min(S, ((x + 63) // 64) * 64)

    NA = r64(max(need[pairs[0][0]], need[pairs[1][0]]))
    NB = r64(max(need[pairs[0][1]], need[pairs[1][1]]))
    return lens, pairs, NA, NB


def _shard_inputs(x_Q, x_K, x_V, src_batch_lens, Wq, bq, Wk, bk, Wv, bv, Wo, bo):
    bf = ml_dtypes.bfloat16
    f32 = np.float32
    lens, pairs, NA, NB = _plan(src_batch_lens)
    NQ = NA + NB

    wq_all = (np.asarray(Wq, f32).transpose(1, 0, 2).reshape(D, H * DH) * SCALE).astype(bf)
    wk_all = np.asarray(Wk, f32).transpose(1, 0, 2).reshape(D, H * DH).astype(bf)
    wv_all = np.asarray(Wv, f32).transpose(1, 0, 2).reshape(D, H * DH).astype(bf)
    bq_all = (np.asarray(bq, f32).reshape(1, H * DH) * SCALE).astype(f32)
    wo_bf = np.asarray(Wo, f32).astype(bf)

    pair_data = []
    for bA, bB in pairs:
        xq = np.zeros((D, NQ), f32)
        m = np.zeros((1, NQ), f32)
        xk = np.empty((D, 2, S), f32)
        xv = np.empty((D, 2, S), f32)
        for slot, (b, off) in enumerate(((bA, 0), (bB, NA))):
            ln = lens[b]
            xq[:, off : off + ln] = np.asarray(x_Q[b], f32).T[:, :ln]
            m[0, off : off + ln] = 1.0
            xk[:, slot, :] = np.asarray(x_K[b], f32).T
            xv[:, slot, :] = np.asarray(x_V[b], f32).T
        pair_data.append(
            (
                np.ascontiguousarray(xq).astype(bf),
                m.astype(bf),
                np.ascontiguousarray(xk).astype(bf),
                np.ascontiguousarray(xv).astype(bf),
            )
        )

    in_maps = []
    for c in range(8):
        p, hq = c // 4, c % 4
        hs = slice(hq * 256, (hq + 1) * 256)
        xqp, mp, xkp, xvp = pair_data[p]
        in_maps.append(
            {
                "xq": xqp,
                "xk": xkp,
                "xv": xvp,
                "wq": np.ascontiguousarray(wq_all[:, hs]),
                "wk": np.ascontiguousarray(wk_all[:, hs]),
                "wv": np.ascontiguousarray(wv_all[:, hs]),
                "wo": np.ascontiguousarray(wo_bf[hs, :]),
                "bq": np.ascontiguousarray(bq_all[:, hs]),
                "mask": mp,
            }
        )
    return in_maps


def kernel(**inputs):
    global _CACHED
    from concourse.bass_utils import run_bass_kernel_spmd

    lens, pairs, NA, NB = _plan(inputs["src_batch_lens"])
    key = (NA, NB)
    if key not in _CACHE:
        _CACHE[key] = _build(NA, NB)
    _CACHED = _CACHE[key]

    in_maps = _shard_inputs(**inputs)
    res = run_bass_kernel_spmd(_CACHED, in_maps, core_ids=list(range(8)))
    # bv folds into an effective output bias: sum_h bv_h @ Wo_h + bo
    bo_eff = (
        np.asarray(bo := inputs["bo"], np.float32)
        + np.asarray(inputs["bv"], np.float32).reshape(-1)
        @ np.asarray(inputs["Wo"], np.float32)
    )
    out = np.empty((B, S, D), np.float32)
    for p, (bA, bB) in enumerate(pairs):
        acc = np.zeros((NA + NB, D), np.float32)
        for hq in range(4):
            acc += np.asarray(res.results[4 * p + hq]["out"], np.float32)
        for b, off in ((bA, 0), (bB, NA)):
            ln = lens[b]
            out[b, :ln] = acc[off : off + ln]
            out[b, ln:] = acc[off + ln]
            out[b] += bo_eff[None, :]
    return out
